# revision 22
# baseline (speedup 1.0000x reference)
"""Block-diagonal compress kernel: out = blockdiag(A) @ W @ blockdiag(B).

Shapes (full): W [8192, 8192] f32, A_blocks [128, 64, 64], B_blocks [128, 64, 64].
Sharding: row-shard W / A over 8 cores (1024 rows = 16 A-blocks each);
B replicated.  Each core computes outT = (A_bd @ W_shard @ B_bd)^T and the
host transposes each shard back on gather.

Per-core dataflow (all sizes per core):
  step 1:  T = (A_bd @ W)^T computed 128-column-chunk-wise with W as the
           matmul *stationary* operand:  matmul(lhsT=W[128 rows, 128 cols],
           rhs=blockdiag(A_even^T, A_odd^T)) -> psum [128 cols, 128 rows].
           This absorbs the transpose that a chained matmul otherwise needs.
  step 2:  outT[chunk] = matmul(lhsT=blockdiag(B_j0, B_j1), rhs=T chunk).

Precision: rel-err budget is 2e-2; W, A, B, T and out are all bf16
(measured pipeline rel err ~3.7e-3) which halves HBM traffic vs f32.

Perf notes (from NTFF traces + the CoreSim cost model):
 - The PE has p-states: 0.65/1.2/2.4 GHz; it only reaches 2.4 GHz after
   ~3us of gap-free execution.  Step 2 of group g-1 is therefore
   interleaved into step 1 of group g so the PE never idles waiting for
   tg copies at group boundaries.
 - GPSIMD cannot touch PSUM, so PSUM->SBUF copies are split into
   [128,512] halves across DVE + ACT (each ~0.62us, two run per slab).
 - DMA descriptors must be ~1MB and DRAM-linear: 256KB descriptors pace
   ~180GB/s, a 4MB descriptor hit 393GB/s.  Aggregate per-core DMA is
   ~400GB/s -> ~35MB of traffic gives a ~90us roofline.
 - Queues: W loads own the sync HWDGE queue; out stores + preloads ride
   the gpsimd SWDGE queue (Pool sequencer is otherwise idle; DVE/ACT
   sequencers must not stall on ~600ns DMA-trigger instructions).
"""

import bass_rust
import numpy as np

import concourse.bass as bass
import concourse.mybir as mybir
from concourse.bass_utils import run_bass_kernel_spmd
from concourse.tile import TileContext

F32 = mybir.dt.float32
BF16 = mybir.dt.bfloat16

N_CORES = 8
D = 8192
BLK = 64
ROWS_PC = D // N_CORES  # 1024 rows of W / out per core

_HOIST_OPCODES = {"Matmult", "DMACopy", "TensorCopy", "Memset", "Activation", "Drain"}


def _hoist_excess_matmul_waits(nc: bass.Bass, max_waits: int = 1) -> None:
    """walrus's codegen for several instruction structs (fused-LDWEIGHTS
    matmul, DMA_DIRECT2D, ...) has few sync-wait slots ("Too many sync wait
    commands"). Move excess semaphore waits off such instructions into
    standalone EventSemaphore instructions right before them on the same
    engine queue — the sequencer executes those in order, so the instruction
    still starts only after all waits pass."""
    ctr = 0
    for fnc in nc.m.functions:
        for bb in fnc.blocks:
            new = []
            for ins in bb.instructions:
                si = ins.sync_info if ins.opcode in _HOIST_OPCODES else None
                if si is not None and len(si.on_wait) > max_waits:
                    waits = list(si.on_wait)
                    for w in waits[:-max_waits]:
                        evs = mybir.InstEventSemaphore(
                            name=f"mmwaithoist-{ctr}", ins=[], outs=[]
                        )
                        ctr += 1
                        evs.engine = ins.engine
                        evs.sync_info = bass_rust.SyncInfo(on_wait=[w], on_update=[])
                        new.append(evs)
                    ins.sync_info.on_wait = waits[-max_waits:]
                new.append(ins)
            bb.instructions[:] = new


def build_nc(rows_pc: int = ROWS_PC, d: int = D, hoist: bool = True) -> bass.Bass:
    """One-core SPMD program."""
    R = rows_pc // 128  # 128-row slabs per core (= A-block pairs); 8
    G = d // 1024       # column groups of 8x128; 8
    assert R == 8 and G == 8

    nc = bass.Bass()
    # W eighths: wq[g*4+hh, p, rl*1024+c] = W_shard[(2hh+rl)*128+p, g*1024+c]
    # -> each descriptor is one fully-linear 512KB HBM read.
    wq_ext = nc.declare_dram_parameter("wq", [4 * G, 128, 2048], BF16, isOutput=False)
    ah_ext = nc.declare_dram_parameter("ah", [128, R * 128], BF16, isOutput=False)
    bp_ext = nc.declare_dram_parameter("bpack", [128, d], BF16, isOutput=False)
    # out quarters: oq[j2//4, p, (j2%4)*1024+i] = outT[j2*128+p, i]
    # -> each store is one fully-linear 1MB HBM write.
    oq_ext = nc.declare_dram_parameter("oq", [d // 512, 128, 4096], BF16, isOutput=True)

    # Whole [128,1024] psum->sbuf copies, one engine each, alternating in a
    # 9:7 ACT:DVE pattern (ACT is ~1.25x faster per element, so it takes the
    # bigger share; per-copy PSUM-access overhead is paid once per unit
    # instead of twice).  At max PE p-state two units of matmuls (~1.7us)
    # outlast one copy (<=1.2us), so copies never block the PE pipeline.
    _PAT = [1, 0, 1, 0, 1, 0, 1, 0, 1, 0, 1, 0, 1, 0, 1, 1]  # 1=ACT, 0=DVE

    def copy_unit(i, out, in_):
        if _PAT[i % 16]:
            nc.scalar.copy(out, in_)
        else:
            nc.vector.tensor_copy(out, in_)

    with TileContext(nc) as tc:
        with (
            tc.tile_pool(name="const", bufs=1) as cpool,
            tc.tile_pool(name="wp", bufs=6) as wpool,
            tc.tile_pool(name="tg", bufs=2) as tpool,
            tc.tile_pool(name="op", bufs=3) as opool,
            tc.tile_pool(name="p1", bufs=2, space="PSUM") as p1pool,
            tc.tile_pool(name="p2", bufs=2, space="PSUM") as p2pool,
        ):
            # ah gates the first matmul: load it FIRST on the fast sync HWDGE
            # queue (delays the first W descriptor by <1us).  bpack is
            # trickled in 256KB chunks INTO the sync queue between early W
            # descriptors — a monolithic 2MB bpack descriptor on any queue
            # monopolizes the 16 shared DMA engines and starves the W stream
            # for ~8us at startup.  Chunk k covers j2 in [8k, 8k+8), needed
            # only by copy-unit ~8k+8, so the deadlines are easy.
            ah = cpool.tile([128, R * 128], BF16)
            nc.sync.dma_start(out=ah[:], in_=ah_ext[:])
            bpack_r = cpool.tile([128, d], BF16)

            tgs = [None, None]  # live tg tiles by group parity
            ots = [None]
            units = [0]  # global copy-unit counter for engine alternation

            def step2_chunk(g, cc):
                """Emit step-2 for column chunk j2 = 8g+cc: 2 matmuls off
                tg(g), psum->sbuf in halves, store every 4th chunk."""
                j2 = 8 * g + cc
                cj = j2 % 4
                tg = tgs[g % 2]
                if cj == 0:
                    ots[0] = opool.tile([128, 4096], BF16, name="ot")
                ot = ots[0]
                p2 = p2pool.tile([128, rows_pc], F32)
                lb = bpack_r[:, j2 * 128 : (j2 + 1) * 128]
                for s in range(2):
                    w0, w1 = s * 512, (s + 1) * 512
                    nc.tensor.matmul(
                        p2[:, w0:w1],
                        lhsT=lb, rhs=tg[:, cc * rows_pc + w0 : cc * rows_pc + w1],
                        start=True, stop=True,
                    )
                copy_unit(units[0], ot[:, cj * 1024 : (cj + 1) * 1024], p2[:])
                units[0] += 1
                qo = j2 // 4
                if qo == 15:
                    # tail fast-drain: store each chunk individually (256KB),
                    # alternating the two now-idle HWDGE queues.
                    eng = nc.sync if cj % 2 == 0 else nc.scalar
                    eng.dma_start(
                        out=oq_ext[qo][:, cj * 1024 : (cj + 1) * 1024],
                        in_=ot[:, cj * 1024 : (cj + 1) * 1024],
                    )
                elif cj == 3:
                    # second-to-last store on the scalar queue so the SWDGE
                    # queue isn't the lone drain at the end.
                    eng = nc.scalar if qo == 14 else nc.gpsimd
                    eng.dma_start(out=oq_ext[qo], in_=ot[:])

            for g in range(G):
                # T for this column group: tg[p, cc*1024 + r*128 + n]
                # = AW^T[g*1024 + cc*128 + p, r*128 + n]
                tg = tpool.tile([128, 8 * rows_pc], BF16)
                tgs[g % 2] = tg
                tgv = tg[:].rearrange("p (cc r n) -> p cc r n", cc=8, r=R)
                for h in range(4):
                    wt = wpool.tile([128, 2048], BF16)
                    nc.sync.dma_start(out=wt[:], in_=wq_ext[4 * g + h])
                    t = 4 * g + h
                    if 2 <= t < 10:  # bpack chunks ride between W descs 2..9
                        k = t - 2
                        nc.sync.dma_start(
                            out=bpack_r[:, k * 1024 : (k + 1) * 1024],
                            in_=bp_ext[:, k * 1024 : (k + 1) * 1024],
                        )
                    for rl in range(2):
                        r = 2 * h + rl
                        p1 = p1pool.tile([128, 1024], F32)
                        rs = slice(r * 128, (r + 1) * 128)
                        for cc in range(8):
                            ws = slice(rl * 1024 + cc * 128, rl * 1024 + (cc + 1) * 128)
                            nc.tensor.matmul(
                                p1[:, cc * 128 : (cc + 1) * 128],
                                lhsT=wt[:, ws], rhs=ah[:, rs],
                                start=True, stop=True,
                            )
                        copy_unit(
                            units[0],
                            tgv[:, :, r, :],
                            p1[:].rearrange("p (cc n) -> p cc n", cc=8),
                        )
                        units[0] += 1
                        # interleave step 2 of the previous group so the PE
                        # never drains at group boundaries (p-state ramp).
                        if g > 0:
                            step2_chunk(g - 1, r)
                for cc in range(8):  # epilogue chunks only for the last group
                    if g == G - 1:
                        step2_chunk(g, cc)
    if hoist:
        _hoist_excess_matmul_waits(nc)
    return nc


def pack_at(a_blocks: np.ndarray) -> np.ndarray:
    """[2R, 64, 64] A blocks -> [128, R*128] with
    out[64*b + k, 128*r + 64*b + n] = A[2r+b][n, k] (transposed, pair-blockdiag)."""
    nb = a_blocks.shape[0]
    R = nb // 2
    out = np.zeros((128, R * 128), np.float32)
    at = a_blocks.transpose(0, 2, 1)
    out[0:64].reshape(64, R, 2, 64)[:, :, 0, :] = at[0::2].transpose(1, 0, 2)
    out[64:128].reshape(64, R, 2, 64)[:, :, 1, :] = at[1::2].transpose(1, 0, 2)
    return out


def pack_b(b_blocks: np.ndarray) -> np.ndarray:
    """[2J, 64, 64] B blocks -> [128, J*128] with
    out[64*b + k, 128*j + 64*b + n] = B[2j+b][k, n] (pair-blockdiag, untransposed)."""
    nb = b_blocks.shape[0]
    J = nb // 2
    out = np.zeros((128, J * 128), np.float32)
    out[0:64].reshape(64, J, 2, 64)[:, :, 0, :] = b_blocks[0::2].transpose(1, 0, 2)
    out[64:128].reshape(64, J, 2, 64)[:, :, 1, :] = b_blocks[1::2].transpose(1, 0, 2)
    return out


def pack_w_q(w_shard: np.ndarray):
    """[1024, 8192] -> bf16 [32, 128, 2048]:
    wq[g*4+hh, p, rl*1024+c] = W[(2hh+rl)*128+p, g*1024+c]."""
    import ml_dtypes

    w5 = w_shard.reshape(4, 2, 128, 8, 1024)  # [hh, rl, p, g, c]
    return np.ascontiguousarray(w5.transpose(3, 0, 2, 1, 4).reshape(32, 128, 2048)).astype(
        ml_dtypes.bfloat16
    )


_NC_CACHE: dict = {}


def run(W, A_blocks, B_blocks, trace: bool = False, trace_cores=None):
    import ml_dtypes

    W = np.asarray(W, dtype=np.float32)
    A_blocks = np.asarray(A_blocks, dtype=np.float32)
    B_blocks = np.asarray(B_blocks, dtype=np.float32)
    assert W.shape == (D, D) and A_blocks.shape == (D // BLK, BLK, BLK)

    if "nc" not in _NC_CACHE:
        _NC_CACHE["nc"] = build_nc()
    nc = _NC_CACHE["nc"]

    bp = pack_b(B_blocks).astype(ml_dtypes.bfloat16)
    in_maps = []
    for c in range(N_CORES):
        wq = pack_w_q(W[ROWS_PC * c : ROWS_PC * (c + 1)])
        ah = pack_at(A_blocks[16 * c : 16 * (c + 1)]).astype(ml_dtypes.bfloat16)
        in_maps.append({"wq": wq, "ah": ah, "bpack": bp})
    res = run_bass_kernel_spmd(nc, in_maps, core_ids=list(range(N_CORES)), trace=trace, trace_cores=trace_cores)
    out = np.empty((D, D), np.float32)
    for c in range(N_CORES):
        oq = np.asarray(res.results[c]["oq"]).reshape(16, 128, 4, 1024)
        out[ROWS_PC * c : ROWS_PC * (c + 1), :] = (
            oq.transpose(3, 0, 2, 1).reshape(ROWS_PC, D).astype(np.float32)
        )
    return out, res


def kernel(W, A_blocks, B_blocks):
    out, _ = run(W, A_blocks, B_blocks, trace=False)
    return out


# revision 27
# speedup vs baseline: 1.0562x; 1.0562x over previous
"""Block-diagonal compress kernel: out = blockdiag(A) @ W @ blockdiag(B).

Shapes (full): W [8192, 8192] f32, A_blocks [128, 64, 64], B_blocks [128, 64, 64].
Sharding: row-shard W / A over 8 cores (1024 rows = 16 A-blocks each);
B replicated.  Each core computes outT = (A_bd @ W_shard @ B_bd)^T and the
host transposes each shard back on gather.

Per-core dataflow (all sizes per core):
  step 1:  T = (A_bd @ W)^T computed 128-column-chunk-wise with W as the
           matmul *stationary* operand:  matmul(lhsT=W[128 rows, 128 cols],
           rhs=blockdiag(A_even^T, A_odd^T)) -> psum [128 cols, 128 rows].
           This absorbs the transpose that a chained matmul otherwise needs.
  step 2:  outT[chunk] = matmul(lhsT=blockdiag(B_j0, B_j1), rhs=T chunk).

Precision: rel-err budget is 2e-2; W, A, B, T and out are all bf16
(measured pipeline rel err ~3.7e-3) which halves HBM traffic vs f32.

Perf notes (from NTFF traces + the CoreSim cost model):
 - The PE has p-states: 0.65/1.2/2.4 GHz; it only reaches 2.4 GHz after
   ~3us of gap-free execution.  Step 2 of group g-1 is therefore
   interleaved into step 1 of group g so the PE never idles waiting for
   tg copies at group boundaries.
 - GPSIMD cannot touch PSUM, so PSUM->SBUF copies are split into
   [128,512] halves across DVE + ACT (each ~0.62us, two run per slab).
 - DMA descriptors must be ~1MB and DRAM-linear: 256KB descriptors pace
   ~180GB/s, a 4MB descriptor hit 393GB/s.  Aggregate per-core DMA is
   ~400GB/s -> ~35MB of traffic gives a ~90us roofline.
 - Queues: W loads own the sync HWDGE queue; out stores + preloads ride
   the gpsimd SWDGE queue (Pool sequencer is otherwise idle; DVE/ACT
   sequencers must not stall on ~600ns DMA-trigger instructions).
"""

import bass_rust
import numpy as np

import concourse.bass as bass
import concourse.mybir as mybir
from concourse.bass_utils import run_bass_kernel_spmd
from concourse.tile import TileContext

F32 = mybir.dt.float32
BF16 = mybir.dt.bfloat16

N_CORES = 8
D = 8192
BLK = 64
ROWS_PC = D // N_CORES  # 1024 rows of W / out per core

_HOIST_OPCODES = {"Matmult", "DMACopy", "TensorCopy", "Memset", "Activation", "Drain"}


def _hoist_excess_matmul_waits(nc: bass.Bass, max_waits: int = 1) -> None:
    """walrus's codegen for several instruction structs (fused-LDWEIGHTS
    matmul, DMA_DIRECT2D, ...) has few sync-wait slots ("Too many sync wait
    commands"). Move excess semaphore waits off such instructions into
    standalone EventSemaphore instructions right before them on the same
    engine queue — the sequencer executes those in order, so the instruction
    still starts only after all waits pass."""
    ctr = 0
    for fnc in nc.m.functions:
        for bb in fnc.blocks:
            new = []
            for ins in bb.instructions:
                si = ins.sync_info if ins.opcode in _HOIST_OPCODES else None
                if si is not None and len(si.on_wait) > max_waits:
                    waits = list(si.on_wait)
                    for w in waits[:-max_waits]:
                        evs = mybir.InstEventSemaphore(
                            name=f"mmwaithoist-{ctr}", ins=[], outs=[]
                        )
                        ctr += 1
                        evs.engine = ins.engine
                        evs.sync_info = bass_rust.SyncInfo(on_wait=[w], on_update=[])
                        new.append(evs)
                    ins.sync_info.on_wait = waits[-max_waits:]
                new.append(ins)
            bb.instructions[:] = new


def build_nc(rows_pc: int = ROWS_PC, d: int = D, hoist: bool = True) -> bass.Bass:
    """One-core SPMD program."""
    R = rows_pc // 128  # 128-row slabs per core (= A-block pairs); 8
    G = d // 1024       # column groups of 8x128; 8
    assert R == 8 and G == 8

    nc = bass.Bass()
    # W halves: wq[g*2+h, p, rl*1024+c] = W_shard[(4h+rl)*128+p, g*1024+c]
    # -> each descriptor is one fully-linear 1MB HBM read (group 0 is pulled
    # as 4 strided 512KB slices instead, for a faster pipeline ramp).
    wq_ext = nc.declare_dram_parameter("wq", [2 * G, 128, 4096], BF16, isOutput=False)
    ah_ext = nc.declare_dram_parameter("ah", [128, R * 128], BF16, isOutput=False)
    bp_ext = nc.declare_dram_parameter("bpack", [128, d], BF16, isOutput=False)
    # out quarters: oq[j2//4, p, (j2%4)*1024+i] = outT[j2*128+p, i]
    # -> each store is one fully-linear 1MB HBM write.
    oq_ext = nc.declare_dram_parameter("oq", [d // 512, 128, 4096], BF16, isOutput=True)

    # Whole [128,1024] psum->sbuf copies, one engine each, alternating in a
    # 9:7 ACT:DVE pattern (ACT is ~1.25x faster per element, so it takes the
    # bigger share; per-copy PSUM-access overhead is paid once per unit
    # instead of twice).  At max PE p-state two units of matmuls (~1.7us)
    # outlast one copy (<=1.2us), so copies never block the PE pipeline.
    _PAT = [1, 0, 1, 0, 1, 0, 1, 0, 1, 0, 1, 0, 1, 0, 1, 1]  # 1=ACT, 0=DVE

    def copy_unit(i, out, in_):
        if _PAT[i % 16]:
            nc.scalar.copy(out, in_)
        else:
            nc.vector.tensor_copy(out, in_)

    with TileContext(nc) as tc:
        with (
            tc.tile_pool(name="const", bufs=1) as cpool,
            tc.tile_pool(name="wps", bufs=4) as wpool_s,
            tc.tile_pool(name="wpl", bufs=6) as wpool_l,
            tc.tile_pool(name="tg", bufs=2) as tpool,
            tc.tile_pool(name="op", bufs=3) as opool,
            tc.tile_pool(name="p1", bufs=2, space="PSUM") as p1pool,
            tc.tile_pool(name="p2", bufs=2, space="PSUM") as p2pool,
        ):
            # ah gates the first matmul: load it FIRST on the fast sync HWDGE
            # queue (delays the first W descriptor by <1us).  bpack is
            # trickled in 256KB chunks INTO the sync queue between early W
            # descriptors — a monolithic 2MB bpack descriptor on any queue
            # monopolizes the 16 shared DMA engines and starves the W stream
            # for ~8us at startup.  Chunk k covers j2 in [8k, 8k+8), needed
            # only by copy-unit ~8k+8, so the deadlines are easy.
            ah = cpool.tile([128, R * 128], BF16)
            nc.sync.dma_start(out=ah[:], in_=ah_ext[:])
            bpack_r = cpool.tile([128, d], BF16)

            tgs = [None, None]  # live tg tiles by group parity
            ots = [None]
            units = [0]  # global copy-unit counter for engine alternation

            def step2_chunk(g, cc):
                """Emit step-2 for column chunk j2 = 8g+cc: 2 matmuls off
                tg(g), psum->sbuf in halves, store every 4th chunk."""
                j2 = 8 * g + cc
                cj = j2 % 4
                tg = tgs[g % 2]
                if cj == 0:
                    ots[0] = opool.tile([128, 4096], BF16, name="ot")
                ot = ots[0]
                p2 = p2pool.tile([128, rows_pc], F32)
                lb = bpack_r[:, j2 * 128 : (j2 + 1) * 128]
                for s in range(2):
                    w0, w1 = s * 512, (s + 1) * 512
                    nc.tensor.matmul(
                        p2[:, w0:w1],
                        lhsT=lb, rhs=tg[:, cc * rows_pc + w0 : cc * rows_pc + w1],
                        start=True, stop=True,
                    )
                copy_unit(units[0], ot[:, cj * 1024 : (cj + 1) * 1024], p2[:])
                units[0] += 1
                qo = j2 // 4
                if qo == 15:
                    # tail fast-drain: store each chunk individually (256KB),
                    # alternating the two now-idle HWDGE queues.
                    eng = nc.sync if cj % 2 == 0 else nc.scalar
                    eng.dma_start(
                        out=oq_ext[qo][:, cj * 1024 : (cj + 1) * 1024],
                        in_=ot[:, cj * 1024 : (cj + 1) * 1024],
                    )
                elif cj == 3:
                    # second-to-last store on the scalar queue so the SWDGE
                    # queue isn't the lone drain at the end.
                    eng = nc.scalar if qo == 14 else nc.gpsimd
                    eng.dma_start(out=oq_ext[qo], in_=ot[:])

            for g in range(G):
                # T for this column group: tg[p, cc*1024 + r*128 + n]
                # = AW^T[g*1024 + cc*128 + p, r*128 + n]
                tg = tpool.tile([128, 8 * rows_pc], BF16)
                tgs[g % 2] = tg
                tgv = tg[:].rearrange("p (cc r n) -> p cc r n", cc=8, r=R)
                # Group 0 loads W as 4x512KB descriptors (fast ramp, with the
                # bpack chunks riding between them); later groups use 2x1MB
                # descriptors (higher stream rate, fewer semaphores).  The
                # deep wpool_l prefetch (~6MB) rides out the transient W
                # starvation when SWDGE store bursts begin (~34us).
                def slab(wt, ws_base, r):
                    p1 = p1pool.tile([128, 1024], F32, name="p1")
                    rs = slice(r * 128, (r + 1) * 128)
                    for cc in range(8):
                        ws = slice(ws_base + cc * 128, ws_base + (cc + 1) * 128)
                        nc.tensor.matmul(
                            p1[:, cc * 128 : (cc + 1) * 128],
                            lhsT=wt[:, ws], rhs=ah[:, rs],
                            start=True, stop=True,
                        )
                    copy_unit(
                        units[0],
                        tgv[:, :, r, :],
                        p1[:].rearrange("p (cc n) -> p cc n", cc=8),
                    )
                    units[0] += 1
                    # interleave step 2 of the previous group so the PE
                    # never drains at group boundaries (p-state ramp).
                    if g > 0:
                        step2_chunk(g - 1, r)

                if g == 0:
                    for t in range(4):
                        h, half = t // 2, t % 2
                        wt = wpool_s.tile([128, 2048], BF16, name="wts")
                        nc.sync.dma_start(
                            out=wt[:],
                            in_=wq_ext[h][:, half * 2048 : (half + 1) * 2048],
                        )
                        if t >= 2:  # first bpack chunks ride between W descs
                            k = t - 2
                            nc.sync.dma_start(
                                out=bpack_r[:, k * 1024 : (k + 1) * 1024],
                                in_=bp_ext[:, k * 1024 : (k + 1) * 1024],
                            )
                        for rl2 in range(2):
                            slab(wt, rl2 * 1024, 4 * h + 2 * half + rl2)
                else:
                    for h in range(2):
                        wt = wpool_l.tile([128, 4096], BF16, name="wtl")
                        nc.sync.dma_start(out=wt[:], in_=wq_ext[2 * g + h])
                        t = 2 * g + h  # 1MB desc index, starts at 2
                        if t < 8:  # remaining bpack chunks between descs
                            k = t
                            nc.sync.dma_start(
                                out=bpack_r[:, k * 1024 : (k + 1) * 1024],
                                in_=bp_ext[:, k * 1024 : (k + 1) * 1024],
                            )
                        for rl in range(4):
                            slab(wt, rl * 1024, 4 * h + rl)
                for cc in range(8):  # epilogue chunks only for the last group
                    if g == G - 1:
                        step2_chunk(g, cc)
    if hoist:
        _hoist_excess_matmul_waits(nc)
    return nc


def pack_at(a_blocks: np.ndarray) -> np.ndarray:
    """[2R, 64, 64] A blocks -> [128, R*128] with
    out[64*b + k, 128*r + 64*b + n] = A[2r+b][n, k] (transposed, pair-blockdiag)."""
    nb = a_blocks.shape[0]
    R = nb // 2
    out = np.zeros((128, R * 128), np.float32)
    at = a_blocks.transpose(0, 2, 1)
    out[0:64].reshape(64, R, 2, 64)[:, :, 0, :] = at[0::2].transpose(1, 0, 2)
    out[64:128].reshape(64, R, 2, 64)[:, :, 1, :] = at[1::2].transpose(1, 0, 2)
    return out


def pack_b(b_blocks: np.ndarray) -> np.ndarray:
    """[2J, 64, 64] B blocks -> [128, J*128] with
    out[64*b + k, 128*j + 64*b + n] = B[2j+b][k, n] (pair-blockdiag, untransposed)."""
    nb = b_blocks.shape[0]
    J = nb // 2
    out = np.zeros((128, J * 128), np.float32)
    out[0:64].reshape(64, J, 2, 64)[:, :, 0, :] = b_blocks[0::2].transpose(1, 0, 2)
    out[64:128].reshape(64, J, 2, 64)[:, :, 1, :] = b_blocks[1::2].transpose(1, 0, 2)
    return out


def pack_w_q(w_shard: np.ndarray):
    """[1024, 8192] -> bf16 [16, 128, 4096]:
    wq[g*2+h, p, rl*1024+c] = W[(4h+rl)*128+p, g*1024+c]."""
    import ml_dtypes

    w5 = w_shard.reshape(2, 4, 128, 8, 1024)  # [h, rl, p, g, c]
    return np.ascontiguousarray(w5.transpose(3, 0, 2, 1, 4).reshape(16, 128, 4096)).astype(
        ml_dtypes.bfloat16
    )


_NC_CACHE: dict = {}


def run(W, A_blocks, B_blocks, trace: bool = False, trace_cores=None):
    import ml_dtypes

    W = np.asarray(W, dtype=np.float32)
    A_blocks = np.asarray(A_blocks, dtype=np.float32)
    B_blocks = np.asarray(B_blocks, dtype=np.float32)
    assert W.shape == (D, D) and A_blocks.shape == (D // BLK, BLK, BLK)

    if "nc" not in _NC_CACHE:
        _NC_CACHE["nc"] = build_nc()
    nc = _NC_CACHE["nc"]

    bp = pack_b(B_blocks).astype(ml_dtypes.bfloat16)
    in_maps = []
    for c in range(N_CORES):
        wq = pack_w_q(W[ROWS_PC * c : ROWS_PC * (c + 1)])
        ah = pack_at(A_blocks[16 * c : 16 * (c + 1)]).astype(ml_dtypes.bfloat16)
        in_maps.append({"wq": wq, "ah": ah, "bpack": bp})
    res = run_bass_kernel_spmd(nc, in_maps, core_ids=list(range(N_CORES)), trace=trace, trace_cores=trace_cores)
    out = np.empty((D, D), np.float32)
    for c in range(N_CORES):
        oq = np.asarray(res.results[c]["oq"]).reshape(16, 128, 4, 1024)
        out[ROWS_PC * c : ROWS_PC * (c + 1), :] = (
            oq.transpose(3, 0, 2, 1).reshape(ROWS_PC, D).astype(np.float32)
        )
    return out, res


def kernel(W, A_blocks, B_blocks):
    out, _ = run(W, A_blocks, B_blocks, trace=False)
    return out


# revision 31
# speedup vs baseline: 1.0769x; 1.0196x over previous
"""Block-diagonal compress kernel: out = blockdiag(A) @ W @ blockdiag(B).

Shapes (full): W [8192, 8192] f32, A_blocks [128, 64, 64], B_blocks [128, 64, 64].
Sharding: row-shard W / A over 8 cores (1024 rows = 16 A-blocks each);
B replicated.  Each core computes outT = (A_bd @ W_shard @ B_bd)^T and the
host transposes each shard back on gather.

Per-core dataflow (all sizes per core):
  step 1:  T = (A_bd @ W)^T computed 128-column-chunk-wise with W as the
           matmul *stationary* operand:  matmul(lhsT=W[128 rows, 128 cols],
           rhs=blockdiag(A_even^T, A_odd^T)) -> psum [128 cols, 128 rows].
           This absorbs the transpose that a chained matmul otherwise needs.
  step 2:  outT[chunk] = matmul(lhsT=blockdiag(B_j0, B_j1), rhs=T chunk).

Precision: rel-err budget is 2e-2; W, A, B, T and out are all bf16
(measured pipeline rel err ~3.7e-3) which halves HBM traffic vs f32.

Perf notes (from NTFF traces + the CoreSim cost model):
 - The PE has p-states: 0.65/1.2/2.4 GHz; it only reaches 2.4 GHz after
   ~3us of gap-free execution.  Step 2 of group g-1 is therefore
   interleaved into step 1 of group g so the PE never idles waiting for
   tg copies at group boundaries.
 - GPSIMD cannot touch PSUM, so PSUM->SBUF copies are split into
   [128,512] halves across DVE + ACT (each ~0.62us, two run per slab).
 - DMA descriptors must be ~1MB and DRAM-linear: 256KB descriptors pace
   ~180GB/s, a 4MB descriptor hit 393GB/s.  Aggregate per-core DMA is
   ~400GB/s -> ~35MB of traffic gives a ~90us roofline.
 - Queues: W loads own the sync HWDGE queue; out stores + preloads ride
   the gpsimd SWDGE queue (Pool sequencer is otherwise idle; DVE/ACT
   sequencers must not stall on ~600ns DMA-trigger instructions).
"""

import bass_rust
import numpy as np

import concourse.bass as bass
import concourse.mybir as mybir
from concourse.bass_utils import run_bass_kernel_spmd
from concourse.tile import TileContext

F32 = mybir.dt.float32
BF16 = mybir.dt.bfloat16

N_CORES = 8
D = 8192
BLK = 64
ROWS_PC = D // N_CORES  # 1024 rows of W / out per core

_HOIST_OPCODES = {"Matmult", "DMACopy", "TensorCopy", "Memset", "Activation", "Drain"}


def _hoist_excess_matmul_waits(nc: bass.Bass, max_waits: int = 1) -> None:
    """walrus's codegen for several instruction structs (fused-LDWEIGHTS
    matmul, DMA_DIRECT2D, ...) has few sync-wait slots ("Too many sync wait
    commands"). Move excess semaphore waits off such instructions into
    standalone EventSemaphore instructions right before them on the same
    engine queue — the sequencer executes those in order, so the instruction
    still starts only after all waits pass."""
    ctr = 0
    for fnc in nc.m.functions:
        for bb in fnc.blocks:
            new = []
            for ins in bb.instructions:
                si = ins.sync_info if ins.opcode in _HOIST_OPCODES else None
                if si is not None and len(si.on_wait) > max_waits:
                    waits = list(si.on_wait)
                    for w in waits[:-max_waits]:
                        evs = mybir.InstEventSemaphore(
                            name=f"mmwaithoist-{ctr}", ins=[], outs=[]
                        )
                        ctr += 1
                        evs.engine = ins.engine
                        evs.sync_info = bass_rust.SyncInfo(on_wait=[w], on_update=[])
                        new.append(evs)
                    ins.sync_info.on_wait = waits[-max_waits:]
                new.append(ins)
            bb.instructions[:] = new


def build_nc(rows_pc: int = ROWS_PC, d: int = D, hoist: bool = True) -> bass.Bass:
    """One-core SPMD program."""
    R = rows_pc // 128  # 128-row slabs per core (= A-block pairs); 8
    G = d // 1024       # column groups of 8x128; 8
    assert R == 8 and G == 8

    nc = bass.Bass()
    # W halves: wq[g*2+h, p, rl*1024+c] = W_shard[(4h+rl)*128+p, g*1024+c]
    # -> each descriptor is one fully-linear 1MB HBM read (group 0 is pulled
    # as 4 strided 512KB slices instead, for a faster pipeline ramp).
    wq_ext = nc.declare_dram_parameter("wq", [2 * G, 128, 4096], BF16, isOutput=False)
    ah_ext = nc.declare_dram_parameter("ah", [128, R * 128], BF16, isOutput=False)
    bp_ext = nc.declare_dram_parameter("bpack", [128, d], BF16, isOutput=False)
    # out quarters: oq[j2//4, p, (j2%4)*1024+i] = outT[j2*128+p, i]
    # -> each store is one fully-linear 1MB HBM write.
    oq_ext = nc.declare_dram_parameter("oq", [d // 512, 128, 4096], BF16, isOutput=True)

    # Whole [128,1024] psum->sbuf copies, one engine each, alternating in a
    # 17:15 ACT:DVE pattern (measured per-copy: ACT 1117ns, DVE 1218ns, so
    # ACT takes a slightly bigger share; per-copy PSUM-access overhead is
    # paid once per unit instead of twice).  At max PE p-state two units of
    # matmuls (~1.7us) outlast one copy (<=1.2us), so copies never block
    # the PE pipeline.
    _PAT = [1, 0, 1, 0, 1, 0, 1, 0, 1, 0, 1, 0, 1, 0, 1, 1] + [1, 0] * 8  # 1=ACT

    def copy_unit(i, out, in_):
        if _PAT[i % 32]:
            nc.scalar.copy(out, in_)
        else:
            nc.vector.tensor_copy(out, in_)

    with TileContext(nc) as tc:
        with (
            tc.tile_pool(name="const", bufs=1) as cpool,
            tc.tile_pool(name="wps", bufs=4) as wpool_s,
            tc.tile_pool(name="wpl", bufs=10) as wpool_l,
            tc.tile_pool(name="tg", bufs=2) as tpool,
            tc.tile_pool(name="op", bufs=4) as opool,
            tc.tile_pool(name="p1", bufs=2, space="PSUM") as p1pool,
            tc.tile_pool(name="p2", bufs=2, space="PSUM") as p2pool,
        ):
            # ah gates the first matmul: load it FIRST on the fast sync HWDGE
            # queue (delays the first W descriptor by <1us).  bpack is
            # trickled in 256KB chunks INTO the sync queue between early W
            # descriptors — a monolithic 2MB bpack descriptor on any queue
            # monopolizes the 16 shared DMA engines and starves the W stream
            # for ~8us at startup.  Chunk k covers j2 in [8k, 8k+8), needed
            # only by copy-unit ~8k+8, so the deadlines are easy.
            ah = cpool.tile([128, R * 128], BF16)
            nc.sync.dma_start(out=ah[:], in_=ah_ext[:])
            bpack_r = cpool.tile([128, d], BF16)

            tgs = [None, None]  # live tg tiles by group parity
            ots = [None]
            units = [0]  # global copy-unit counter for engine alternation

            def step2_chunk(g, cc):
                """Emit step-2 for column chunk j2 = 8g+cc: 2 matmuls off
                tg(g), psum->sbuf in halves, store every 4th chunk."""
                j2 = 8 * g + cc
                cj = j2 % 4
                tg = tgs[g % 2]
                if cj == 0:
                    ots[0] = opool.tile([128, 4096], BF16, name="ot")
                ot = ots[0]
                p2 = p2pool.tile([128, rows_pc], F32)
                lb = bpack_r[:, j2 * 128 : (j2 + 1) * 128]
                for s in range(2):
                    w0, w1 = s * 512, (s + 1) * 512
                    nc.tensor.matmul(
                        p2[:, w0:w1],
                        lhsT=lb, rhs=tg[:, cc * rows_pc + w0 : cc * rows_pc + w1],
                        start=True, stop=True,
                    )
                copy_unit(units[0], ot[:, cj * 1024 : (cj + 1) * 1024], p2[:])
                units[0] += 1
                qo = j2 // 4
                if qo == 15:
                    # tail fast-drain: store each chunk individually (256KB),
                    # alternating the two now-idle HWDGE queues.
                    eng = nc.sync if cj % 2 == 0 else nc.scalar
                    eng.dma_start(
                        out=oq_ext[qo][:, cj * 1024 : (cj + 1) * 1024],
                        in_=ot[:, cj * 1024 : (cj + 1) * 1024],
                    )
                elif cj == 3:
                    # second-to-last store on the scalar queue so the SWDGE
                    # queue isn't the lone drain at the end.
                    eng = nc.scalar if qo == 14 else nc.gpsimd
                    eng.dma_start(out=oq_ext[qo], in_=ot[:])

            for g in range(G):
                # T for this column group: tg[p, cc*1024 + r*128 + n]
                # = AW^T[g*1024 + cc*128 + p, r*128 + n]
                tg = tpool.tile([128, 8 * rows_pc], BF16)
                tgs[g % 2] = tg
                tgv = tg[:].rearrange("p (cc r n) -> p cc r n", cc=8, r=R)
                # Group 0 loads W as 4x512KB descriptors (fast ramp, with the
                # bpack chunks riding between them); later groups use 2x1MB
                # descriptors (higher stream rate, fewer semaphores).  The
                # deep wpool_l prefetch (~6MB) rides out the transient W
                # starvation when SWDGE store bursts begin (~34us).
                def slab(wt, ws_base, r):
                    p1 = p1pool.tile([128, 1024], F32, name="p1")
                    rs = slice(r * 128, (r + 1) * 128)
                    for cc in range(8):
                        ws = slice(ws_base + cc * 128, ws_base + (cc + 1) * 128)
                        nc.tensor.matmul(
                            p1[:, cc * 128 : (cc + 1) * 128],
                            lhsT=wt[:, ws], rhs=ah[:, rs],
                            start=True, stop=True,
                        )
                    copy_unit(
                        units[0],
                        tgv[:, :, r, :],
                        p1[:].rearrange("p (cc n) -> p cc n", cc=8),
                    )
                    units[0] += 1
                    # interleave step 2 of the previous group so the PE
                    # never drains at group boundaries (p-state ramp).
                    if g > 0:
                        step2_chunk(g - 1, r)

                if g == 0:
                    for t in range(4):
                        h, half = t // 2, t % 2
                        wt = wpool_s.tile([128, 2048], BF16, name="wts")
                        nc.sync.dma_start(
                            out=wt[:],
                            in_=wq_ext[h][:, half * 2048 : (half + 1) * 2048],
                        )
                        if t >= 2:  # first bpack chunks ride between W descs
                            k = t - 2
                            nc.sync.dma_start(
                                out=bpack_r[:, k * 1024 : (k + 1) * 1024],
                                in_=bp_ext[:, k * 1024 : (k + 1) * 1024],
                            )
                        for rl2 in range(2):
                            slab(wt, rl2 * 1024, 4 * h + 2 * half + rl2)
                else:
                    for h in range(2):
                        wt = wpool_l.tile([128, 4096], BF16, name="wtl")
                        nc.sync.dma_start(out=wt[:], in_=wq_ext[2 * g + h])
                        t = 2 * g + h  # 1MB desc index, starts at 2
                        if t < 8:  # remaining bpack chunks between descs
                            k = t
                            nc.sync.dma_start(
                                out=bpack_r[:, k * 1024 : (k + 1) * 1024],
                                in_=bp_ext[:, k * 1024 : (k + 1) * 1024],
                            )
                        for rl in range(4):
                            slab(wt, rl * 1024, 4 * h + rl)
                for cc in range(8):  # epilogue chunks only for the last group
                    if g == G - 1:
                        step2_chunk(g, cc)
    if hoist:
        _hoist_excess_matmul_waits(nc)
    return nc


def pack_at(a_blocks: np.ndarray) -> np.ndarray:
    """[2R, 64, 64] A blocks -> [128, R*128] with
    out[64*b + k, 128*r + 64*b + n] = A[2r+b][n, k] (transposed, pair-blockdiag)."""
    nb = a_blocks.shape[0]
    R = nb // 2
    out = np.zeros((128, R * 128), np.float32)
    at = a_blocks.transpose(0, 2, 1)
    out[0:64].reshape(64, R, 2, 64)[:, :, 0, :] = at[0::2].transpose(1, 0, 2)
    out[64:128].reshape(64, R, 2, 64)[:, :, 1, :] = at[1::2].transpose(1, 0, 2)
    return out


def pack_b(b_blocks: np.ndarray) -> np.ndarray:
    """[2J, 64, 64] B blocks -> [128, J*128] with
    out[64*b + k, 128*j + 64*b + n] = B[2j+b][k, n] (pair-blockdiag, untransposed)."""
    nb = b_blocks.shape[0]
    J = nb // 2
    out = np.zeros((128, J * 128), np.float32)
    out[0:64].reshape(64, J, 2, 64)[:, :, 0, :] = b_blocks[0::2].transpose(1, 0, 2)
    out[64:128].reshape(64, J, 2, 64)[:, :, 1, :] = b_blocks[1::2].transpose(1, 0, 2)
    return out


def pack_w_q(w_shard: np.ndarray):
    """[1024, 8192] -> bf16 [16, 128, 4096]:
    wq[g*2+h, p, rl*1024+c] = W[(4h+rl)*128+p, g*1024+c]."""
    import ml_dtypes

    w5 = w_shard.reshape(2, 4, 128, 8, 1024)  # [h, rl, p, g, c]
    return np.ascontiguousarray(w5.transpose(3, 0, 2, 1, 4).reshape(16, 128, 4096)).astype(
        ml_dtypes.bfloat16
    )


_NC_CACHE: dict = {}


def run(W, A_blocks, B_blocks, trace: bool = False, trace_cores=None):
    import ml_dtypes

    W = np.asarray(W, dtype=np.float32)
    A_blocks = np.asarray(A_blocks, dtype=np.float32)
    B_blocks = np.asarray(B_blocks, dtype=np.float32)
    assert W.shape == (D, D) and A_blocks.shape == (D // BLK, BLK, BLK)

    if "nc" not in _NC_CACHE:
        _NC_CACHE["nc"] = build_nc()
    nc = _NC_CACHE["nc"]

    bp = pack_b(B_blocks).astype(ml_dtypes.bfloat16)
    in_maps = []
    for c in range(N_CORES):
        wq = pack_w_q(W[ROWS_PC * c : ROWS_PC * (c + 1)])
        ah = pack_at(A_blocks[16 * c : 16 * (c + 1)]).astype(ml_dtypes.bfloat16)
        in_maps.append({"wq": wq, "ah": ah, "bpack": bp})
    res = run_bass_kernel_spmd(nc, in_maps, core_ids=list(range(N_CORES)), trace=trace, trace_cores=trace_cores)
    out = np.empty((D, D), np.float32)
    for c in range(N_CORES):
        oq = np.asarray(res.results[c]["oq"]).reshape(16, 128, 4, 1024)
        out[ROWS_PC * c : ROWS_PC * (c + 1), :] = (
            oq.transpose(3, 0, 2, 1).reshape(ROWS_PC, D).astype(np.float32)
        )
    return out, res


def kernel(W, A_blocks, B_blocks):
    out, _ = run(W, A_blocks, B_blocks, trace=False)
    return out
